# revision 19
# baseline (speedup 1.0000x reference)
"""Trainium2 Bass kernel for nn_GroupLocalSL2 — 23-stream variant.

out[b,o,i,xo,yo] = sum_{c,f,kh,kw} x[b,c,idx[i,f],xo+kh,yo+kw] * W[o,c,f,kh,kw] + bias[o]

Same skeleton as kernel.py (B=8 data-parallel, (c,f)-in-K, kw-pairs in M,
kh via PSUM accumulation), but the B-chunk (f=4..6, 96 rows) is repacked so
every B matmul streams a FULL 128-row contraction:

  The 15 B work-units (f in {4,5,6}) x (kh in 0..4), 32 rows each, pack into
  4 streams of 4 units instead of 5 streams of 3. Stream delta reads x rows
  r0+delta; a unit (f, kh) rides it iff its plane data is present shifted by
  (kh - delta) in {0, +1}. The +1-shifted copies are plain DMA loads of the
  same pre-gathered DRAM planes with a +1 source-row offset — no on-chip
  shuffles. Unit->stream map (fb = f-4):
     d0: (0,0)(1,0)(2,0)(0,1)   d1: (1,1)(2,1)(0,2)(1,2)
     d2: (2,2)(0,3)(1,3)(2,3)   d3: (0,4)(1,4)(2,4)(spare, zero weights)
  Phase-1 drops from 20 to 18 streams per row-chunk (25 -> 23 streams per
  output px; bf16 MAC floor is 21.875): ~45us less TensorE streaming.
  Cost: x-plane HBM traffic rises 2.8x (5.3MB/group), split across both
  HWDGE rings (sync: xa,xb0,xb1; scalar: xb2,xb3) so prefetch stays hidden.
"""

import os
import sys

import numpy as np
import ml_dtypes

for _p in ("/opt/trn_rl_repo", "/root/.axon_site/_ro/trn_rl_repo"):
    if os.path.isdir(_p) and _p not in sys.path:
        sys.path.append(_p)

import concourse.bass as bass
import concourse.mybir as mybir
import concourse.tile as tile
from concourse import bacc
from concourse.bass_utils import run_bass_kernel_spmd

BF16 = ml_dtypes.bfloat16

B, C, G_IN = 8, 32, 33
O, G_F, KH, KW = 64, 7, 5, 5
X, Y = 64, 64
G_OUT = 15
XO, YO = X - KH + 1, Y - KW + 1  # 60, 60
RCH = 8  # output rows per chunk (8*61 = 488 <= 512 psum bank)
N_WARM = 20  # dummy matmuls bridging program start to group-0 data-ready

# B-chunk unit map: BMAP[delta][slot] = (fb, kh); plane f=4+fb shifted by
# (kh - delta) rows lives at partitions slot*32:(slot+1)*32 of tile delta.
BMAP = {
    0: [(0, 0), (1, 0), (2, 0), (0, 1)],
    1: [(1, 1), (2, 1), (0, 2), (1, 2)],
    2: [(2, 2), (0, 3), (1, 3), (2, 3)],
    3: [(0, 4), (1, 4), (2, 4), None],
}


def _build_nc(n_groups=G_OUT):
    """Build the single-core Bass program (x pre-gathered host-side)."""
    nc = bacc.Bacc("TRN2", target_bir_lowering=False, debug=False)
    dt = mybir.dt
    ga_d = nc.dram_tensor("ga", [G_OUT, 128, X, Y + 1], dt.bfloat16, kind="ExternalInput")
    # the four B-tiles fully materialized host-side (shifted copies baked
    # in) so each is ONE contiguous DMA — HWDGE pays ~1.3us fixed per
    # transfer, so fewer/bigger transfers shorten the group-0 fill
    gb_d = nc.dram_tensor("gb", [G_OUT, 4, 128, X, Y + 1], dt.bfloat16, kind="ExternalInput")
    # packed weights: rows 0:5 = wa[kh], rows 5:10 = wb[kh] (kw4 odd half),
    # rows 10:14 = B-stream weights per delta (cols 0:256, kw-pair blocks)
    wt_d = nc.dram_tensor("wt", [128, 14, KW * O], dt.bfloat16, kind="ExternalInput")
    bias_d = nc.dram_tensor("bias", [O, 1], dt.float32, kind="ExternalInput")
    out_d = nc.dram_tensor("out", [O, G_OUT, XO, YO], dt.float32, kind="ExternalOutput")

    rchunks = [(r0, min(RCH, XO - r0)) for r0 in range(0, XO, RCH)]

    with tile.TileContext(nc) as tc:
        with (
            tc.tile_pool(name="wpool", bufs=1) as wpool,
            tc.tile_pool(name="warm", bufs=1) as warmpool,
            tc.tile_pool(name="xpool", bufs=2) as xpool,
            tc.tile_pool(name="tpool", bufs=3) as tpool,
            tc.tile_pool(name="opool", bufs=4) as opool,
            tc.tile_pool(name="psum", bufs=7, space="PSUM") as pp,
            tc.tile_pool(name="psumt", bufs=1, space="PSUM") as ppt,
        ):
            wmt = warmpool.tile([128, 256], dt.bfloat16, tag="warm")
            nc.vector.memset(wmt[:, :], 0.0)

            wt = wpool.tile([128, 14, KW * O], dt.bfloat16, tag="wt")
            bias_sb = wpool.tile([O, 1], dt.float32, tag="bias")
            nc.scalar.dma_start(wt[:, :, 0:128], wt_d[:, :, 0:128])
            nc.scalar.dma_start(wt[:, :, 128:320], wt_d[:, :, 128:320])
            nc.scalar.dma_start(bias_sb[:, :], bias_d[:, :])

            for i in range(n_groups):
                xa = xpool.tile([128, X, Y + 1], dt.bfloat16, tag="xa")
                xbs = [
                    xpool.tile(
                        [128, X, Y + 1],
                        dt.bfloat16,
                        tag="xb%d" % d,
                        name="xb%d" % d,
                    )
                    for d in range(4)
                ]
                # one contiguous DMA per tile, split across both HWDGE rings;
                # group 0 in three row bands in consumption order so the
                # first row chunks' matmuls start ~15us earlier
                bands = ((0, 16), (16, 40), (40, X)) if i == 0 else ((0, X),)
                for lo, hi in bands:
                    nc.sync.dma_start(xa[:, lo:hi, :], ga_d[i, :, lo:hi, :])
                    nc.sync.dma_start(xbs[0][:, lo:hi, :], gb_d[i, 0, :, lo:hi, :])
                    nc.sync.dma_start(xbs[1][:, lo:hi, :], gb_d[i, 1, :, lo:hi, :])
                    nc.scalar.dma_start(xbs[2][:, lo:hi, :], gb_d[i, 2, :, lo:hi, :])
                    nc.scalar.dma_start(xbs[3][:, lo:hi, :], gb_d[i, 3, :, lo:hi, :])
                if i == 0:
                    wps = ppt.tile([128, 4, 61], dt.float32, tag="pt")
                    for _ in range(N_WARM):
                        nc.tensor.matmul(
                            wps[:, :, :],
                            wmt[:, 0:128],
                            wmt[:, 0:244],
                            start=True,
                            stop=True,
                        )

                # Phase 1: per row chunk, 2 kw-pair groups x (5 A-streams +
                # 4 B-streams), all M=128 K=128, one psum bank per chunk.
                ptiles = []
                for r0, R in rchunks:
                    tail = R != RCH
                    p = (ppt if tail else pp).tile(
                        [128, R, 61], dt.float32, tag="pt" if tail else "p"
                    )
                    ptiles.append(p)
                    for grp in (0, 1):
                        c0 = 2 * grp
                        for kh in range(KH):
                            nc.tensor.matmul(
                                p[:, 0:R, :],
                                wt[0:128, kh, grp * 128 : grp * 128 + 128],
                                xa[0:128, r0 + kh : r0 + kh + R, c0 : c0 + 61],
                                start=(grp == 0 and kh == 0),
                                stop=False,
                            )
                        for dlt in range(4):
                            nc.tensor.matmul(
                                p[:, 0:R, :],
                                wt[0:128, 10 + dlt, grp * 128 : grp * 128 + 128],
                                xbs[dlt][0:128, r0 + dlt : r0 + dlt + R, c0 : c0 + 61],
                                start=False,
                                stop=False,
                            )
                for ri, ((r0, R), p) in enumerate(zip(rchunks, ptiles)):
                    # kw4 col-tiled pair (even: xa cols 4:65; odd: natural
                    # f4-6 at xb0[0:96], cols 3:64)
                    for kh in range(KH):
                        nc.tensor.matmul(
                            p[0:64, 0:R, :],
                            wt[0:128, kh, 256:320],
                            xa[0:128, r0 + kh : r0 + kh + R, 4:65],
                            start=False,
                            stop=False,
                        )
                        nc.tensor.matmul(
                            p[64:128, 0:R, :],
                            wt[0:96, KH + kh, 256:320],
                            xbs[0][0:96, r0 + kh : r0 + kh + R, 3:64],
                            start=False,
                            stop=(kh == KH - 1),
                        )

                    t = tpool.tile([O, RCH, 60], dt.float32, tag="t")
                    ot = opool.tile([O, RCH, 60], dt.float32, tag="out")
                    nc.scalar.add(t[:, 0:R, :], p[0:64, 0:R, 0:60], bias_sb[:, 0:1])
                    nc.vector.tensor_add(
                        ot[:, 0:R, :], t[:, 0:R, :], p[64:128, 0:R, 1:61]
                    )
                    if i == n_groups - 1 and ri == len(rchunks) - 1:
                        rh = max(R // 2, 1)
                        nc.sync.dma_start(
                            out_d[:, i, r0 : r0 + rh, :], ot[:, 0:rh, :]
                        )
                        nc.scalar.dma_start(
                            out_d[:, i, r0 + rh : r0 + R, :], ot[:, rh:R, :]
                        )
                    else:
                        nc.sync.dma_start(
                            out_d[:, i, r0 : r0 + R, :], ot[:, 0:R, :]
                        )
    nc.compile()
    return nc


def _prep_inputs(x, weight, bias, idx):
    """Host-side staging: bf16 cast, idx gather, packed lhsT weights."""
    x16 = np.asarray(x).astype(BF16)  # [B, C, G_IN, X, Y]
    x16 = np.pad(x16, ((0, 0), (0, 0), (0, 0), (0, 0), (0, 1)))
    w = np.asarray(weight).astype(np.float32)
    wx = w.transpose(2, 1, 3, 4, 0)  # [G_F, C, KH, KW, O]
    wa = wx[0:4].reshape(128, KH, KW * O)
    wb = wx[4:7].reshape(96, KH, KW * O)
    wfull = np.zeros((128, 14, KW * O), dtype=np.float32)
    wfull[:, 0:KH] = wa
    wfull[0:96, KH : 2 * KH] = wb
    for d, units in BMAP.items():
        for s, u in enumerate(units):
            if u is None:
                continue
            fb, kh = u
            wfull[32 * s : 32 * (s + 1), 10 + d, 0:256] = wx[
                4 + fb, :, kh, 0:4, :
            ].reshape(32, 256)
    wfull = np.ascontiguousarray(wfull).astype(BF16)
    b2 = np.ascontiguousarray(np.asarray(bias).astype(np.float32).reshape(O, 1))
    in_maps = []
    for b in range(B):
        gx = x16[b][:, idx]  # [C, G_OUT, G_F, X, Y+1]
        gx = gx.transpose(1, 2, 0, 3, 4).reshape(G_OUT, G_F * C, X, Y + 1)
        # materialize the four B-tiles (BMAP slots, shifts baked in)
        gb23 = np.zeros((G_OUT, 4, 128, X, Y + 1), dtype=BF16)
        for d, units in BMAP.items():
            for s, u in enumerate(units):
                if u is None:
                    continue
                fb, kh = u
                sh = kh - d
                gb23[:, d, 32 * s : 32 * (s + 1), 0 : X - sh] = gx[
                    :, 128 + 32 * fb : 128 + 32 * (fb + 1), sh:X
                ]
        in_maps.append(
            {
                "ga": np.ascontiguousarray(gx[:, 0:128]),
                "gb": np.ascontiguousarray(gb23),
                "wt": wfull,
                "bias": b2,
            }
        )
    return in_maps


def run(x, weight, bias, idx, trace=False):
    idx = np.asarray(idx).astype(np.int64)
    assert idx.shape == (G_OUT, G_F) and idx.min() >= 0 and idx.max() < G_IN
    nc = _build_nc()
    in_maps = _prep_inputs(x, weight, bias, idx)
    res = run_bass_kernel_spmd(nc, in_maps, list(range(B)), trace=trace)
    out = np.stack([res.results[b]["out"] for b in range(B)]).astype(np.float32)
    return out, res


def kernel(x, weight, bias, idx):
    out, _ = run(x, weight, bias, idx, trace=False)
    return out


# revision 21
# speedup vs baseline: 1.1900x; 1.1900x over previous
"""Trainium2 Bass kernel for nn_GroupLocalSL2 — 23-stream variant.

out[b,o,i,xo,yo] = sum_{c,f,kh,kw} x[b,c,idx[i,f],xo+kh,yo+kw] * W[o,c,f,kh,kw] + bias[o]

Same skeleton as kernel.py (B=8 data-parallel, (c,f)-in-K, kw-pairs in M,
kh via PSUM accumulation), but the B-chunk (f=4..6, 96 rows) is repacked so
every B matmul streams a FULL 128-row contraction:

  The 15 B work-units (f in {4,5,6}) x (kh in 0..4), 32 rows each, pack into
  4 streams of 4 units instead of 5 streams of 3. Stream delta reads x rows
  r0+delta; a unit (f, kh) rides it iff its plane data is present shifted by
  (kh - delta) in {0, +1}. The +1-shifted copies are plain DMA loads of the
  same pre-gathered DRAM planes with a +1 source-row offset — no on-chip
  shuffles. Unit->stream map (fb = f-4):
     d0: (0,0)(1,0)(2,0)(0,1)   d1: (1,1)(2,1)(0,2)(1,2)
     d2: (2,2)(0,3)(1,3)(2,3)   d3: (0,4)(1,4)(2,4)(spare, zero weights)
  Phase-1 drops from 20 to 18 streams per row-chunk (25 -> 23 streams per
  output px; bf16 MAC floor is 21.875): ~45us less TensorE streaming.
  Cost: x-plane HBM traffic rises 2.8x (5.3MB/group), split across both
  HWDGE rings (sync: xa,xb0,xb1; scalar: xb2,xb3) so prefetch stays hidden.
"""

import os
import sys

import numpy as np
import ml_dtypes

for _p in ("/opt/trn_rl_repo", "/root/.axon_site/_ro/trn_rl_repo"):
    if os.path.isdir(_p) and _p not in sys.path:
        sys.path.append(_p)

import concourse.bass as bass
import concourse.mybir as mybir
import concourse.tile as tile
from concourse import bacc
from concourse.bass_utils import run_bass_kernel_spmd

BF16 = ml_dtypes.bfloat16

B, C, G_IN = 8, 32, 33
O, G_F, KH, KW = 64, 7, 5, 5
X, Y = 64, 64
G_OUT = 15
XO, YO = X - KH + 1, Y - KW + 1  # 60, 60
RCH = 8  # output rows per chunk (8*61 = 488 <= 512 psum bank)
N_WARM = 22  # dummy matmuls bridging program start to group-0 data-ready

# B-chunk unit map: BMAP[delta][slot] = (fb, kh); plane f=4+fb shifted by
# (kh - delta) rows lives at partitions slot*32:(slot+1)*32 of tile delta.
BMAP = {
    0: [(0, 0), (1, 0), (2, 0), (0, 1)],
    1: [(1, 1), (2, 1), (0, 2), (1, 2)],
    2: [(2, 2), (0, 3), (1, 3), (2, 3)],
    3: [(0, 4), (1, 4), (2, 4), None],
}


def _build_nc(n_groups=G_OUT):
    """Build the single-core Bass program (x pre-gathered host-side)."""
    nc = bacc.Bacc("TRN2", target_bir_lowering=False, debug=False)
    dt = mybir.dt
    ga_d = nc.dram_tensor("ga", [G_OUT, 128, X, Y + 1], dt.bfloat16, kind="ExternalInput")
    # the four B-tiles fully materialized host-side (shifted copies baked
    # in) so each is ONE contiguous DMA — HWDGE pays ~1.3us fixed per
    # transfer, so fewer/bigger transfers shorten the group-0 fill
    gb_d = nc.dram_tensor("gb", [G_OUT, 4, 128, X, Y + 1], dt.bfloat16, kind="ExternalInput")
    # packed weights: rows 0:5 = wa[kh], rows 5:10 = wb[kh] (kw4 odd half),
    # rows 10:14 = B-stream weights per delta (cols 0:256, kw-pair blocks)
    wt_d = nc.dram_tensor("wt", [128, 14, KW * O], dt.bfloat16, kind="ExternalInput")
    bias_d = nc.dram_tensor("bias", [O, 1], dt.float32, kind="ExternalInput")
    out_d = nc.dram_tensor("out", [O, G_OUT, XO, YO], dt.float32, kind="ExternalOutput")

    rchunks = [(r0, min(RCH, XO - r0)) for r0 in range(0, XO, RCH)]

    with tile.TileContext(nc) as tc:
        with (
            tc.tile_pool(name="wpool", bufs=1) as wpool,
            tc.tile_pool(name="warm", bufs=1) as warmpool,
            tc.tile_pool(name="xpool", bufs=2) as xpool,
            tc.tile_pool(name="tpool", bufs=3) as tpool,
            tc.tile_pool(name="opool", bufs=4) as opool,
            tc.tile_pool(name="psum", bufs=7, space="PSUM") as pp,
            tc.tile_pool(name="psumt", bufs=1, space="PSUM") as ppt,
        ):
            wmt = warmpool.tile([128, 256], dt.bfloat16, tag="warm")
            nc.gpsimd.memset(wmt[:, :], 0.0)

            # both kw-pair weight blocks in the FIRST transfer (matmul #10+
            # needs cols 128:256 just ~0.5us after #1 needs 0:128 — a split
            # there stalls the stream ~1.2us); kw4 block second, bias last
            wt = wpool.tile([128, 14, KW * O], dt.bfloat16, tag="wt")
            bias_sb = wpool.tile([O, 1], dt.float32, tag="bias")
            nc.scalar.dma_start(wt[:, :, 0:256], wt_d[:, :, 0:256])
            nc.scalar.dma_start(wt[:, :, 256:320], wt_d[:, :, 256:320])
            nc.scalar.dma_start(bias_sb[:, :], bias_d[:, :])

            for i in range(n_groups):
                xa = xpool.tile([128, X, Y + 1], dt.bfloat16, tag="xa")
                xbs = [
                    xpool.tile(
                        [128, X, Y + 1],
                        dt.bfloat16,
                        tag="xb%d" % d,
                        name="xb%d" % d,
                    )
                    for d in range(4)
                ]
                # one contiguous DMA per tile, split across both HWDGE rings;
                # group 0 in three row bands in consumption order so the
                # first row chunks' matmuls start ~15us earlier
                bands = ((0, 16), (16, 40), (40, X)) if i == 0 else ((0, X),)
                for lo, hi in bands:
                    nc.sync.dma_start(xa[:, lo:hi, :], ga_d[i, :, lo:hi, :])
                    nc.sync.dma_start(xbs[0][:, lo:hi, :], gb_d[i, 0, :, lo:hi, :])
                    nc.sync.dma_start(xbs[1][:, lo:hi, :], gb_d[i, 1, :, lo:hi, :])
                    nc.scalar.dma_start(xbs[2][:, lo:hi, :], gb_d[i, 2, :, lo:hi, :])
                    nc.scalar.dma_start(xbs[3][:, lo:hi, :], gb_d[i, 3, :, lo:hi, :])
                if i == 0:
                    wps = ppt.tile([128, 4, 61], dt.float32, tag="pt")
                    for _ in range(N_WARM):
                        nc.tensor.matmul(
                            wps[:, :, :],
                            wmt[:, 0:128],
                            wmt[:, 0:244],
                            start=True,
                            stop=True,
                        )

                # Phase 1: per row chunk, 2 kw-pair groups x (5 A-streams +
                # 4 B-streams), all M=128 K=128, one psum bank per chunk.
                ptiles = []
                for r0, R in rchunks:
                    tail = R != RCH
                    p = (ppt if tail else pp).tile(
                        [128, R, 61], dt.float32, tag="pt" if tail else "p"
                    )
                    ptiles.append(p)
                    for grp in (0, 1):
                        c0 = 2 * grp
                        for kh in range(KH):
                            nc.tensor.matmul(
                                p[:, 0:R, :],
                                wt[0:128, kh, grp * 128 : grp * 128 + 128],
                                xa[0:128, r0 + kh : r0 + kh + R, c0 : c0 + 61],
                                start=(grp == 0 and kh == 0),
                                stop=False,
                            )
                        for dlt in range(4):
                            nc.tensor.matmul(
                                p[:, 0:R, :],
                                wt[0:128, 10 + dlt, grp * 128 : grp * 128 + 128],
                                xbs[dlt][0:128, r0 + dlt : r0 + dlt + R, c0 : c0 + 61],
                                start=False,
                                stop=False,
                            )
                for ri, ((r0, R), p) in enumerate(zip(rchunks, ptiles)):
                    # kw4 col-tiled pair (even: xa cols 4:65; odd: natural
                    # f4-6 at xb0[0:96], cols 3:64)
                    for kh in range(KH):
                        nc.tensor.matmul(
                            p[0:64, 0:R, :],
                            wt[0:128, kh, 256:320],
                            xa[0:128, r0 + kh : r0 + kh + R, 4:65],
                            start=False,
                            stop=False,
                        )
                        nc.tensor.matmul(
                            p[64:128, 0:R, :],
                            wt[0:96, KH + kh, 256:320],
                            xbs[0][0:96, r0 + kh : r0 + kh + R, 3:64],
                            start=False,
                            stop=(kh == KH - 1),
                        )

                    t = tpool.tile([O, RCH, 60], dt.float32, tag="t")
                    ot = opool.tile([O, RCH, 60], dt.float32, tag="out")
                    nc.scalar.add(t[:, 0:R, :], p[0:64, 0:R, 0:60], bias_sb[:, 0:1])
                    nc.vector.tensor_add(
                        ot[:, 0:R, :], t[:, 0:R, :], p[64:128, 0:R, 1:61]
                    )
                    if i == n_groups - 1 and ri == len(rchunks) - 1:
                        rh = max(R // 2, 1)
                        nc.sync.dma_start(
                            out_d[:, i, r0 : r0 + rh, :], ot[:, 0:rh, :]
                        )
                        nc.scalar.dma_start(
                            out_d[:, i, r0 + rh : r0 + R, :], ot[:, rh:R, :]
                        )
                    else:
                        nc.sync.dma_start(
                            out_d[:, i, r0 : r0 + R, :], ot[:, 0:R, :]
                        )
    nc.compile()
    return nc


def _prep_inputs(x, weight, bias, idx):
    """Host-side staging: bf16 cast, idx gather, packed lhsT weights."""
    x16 = np.asarray(x).astype(BF16)  # [B, C, G_IN, X, Y]
    x16 = np.pad(x16, ((0, 0), (0, 0), (0, 0), (0, 0), (0, 1)))
    w = np.asarray(weight).astype(np.float32)
    wx = w.transpose(2, 1, 3, 4, 0)  # [G_F, C, KH, KW, O]
    wa = wx[0:4].reshape(128, KH, KW * O)
    wb = wx[4:7].reshape(96, KH, KW * O)
    wfull = np.zeros((128, 14, KW * O), dtype=np.float32)
    wfull[:, 0:KH] = wa
    wfull[0:96, KH : 2 * KH] = wb
    for d, units in BMAP.items():
        for s, u in enumerate(units):
            if u is None:
                continue
            fb, kh = u
            wfull[32 * s : 32 * (s + 1), 10 + d, 0:256] = wx[
                4 + fb, :, kh, 0:4, :
            ].reshape(32, 256)
    wfull = np.ascontiguousarray(wfull).astype(BF16)
    b2 = np.ascontiguousarray(np.asarray(bias).astype(np.float32).reshape(O, 1))
    in_maps = []
    for b in range(B):
        gx = x16[b][:, idx]  # [C, G_OUT, G_F, X, Y+1]
        gx = gx.transpose(1, 2, 0, 3, 4).reshape(G_OUT, G_F * C, X, Y + 1)
        # materialize the four B-tiles (BMAP slots, shifts baked in)
        gb23 = np.zeros((G_OUT, 4, 128, X, Y + 1), dtype=BF16)
        for d, units in BMAP.items():
            for s, u in enumerate(units):
                if u is None:
                    continue
                fb, kh = u
                sh = kh - d
                gb23[:, d, 32 * s : 32 * (s + 1), 0 : X - sh] = gx[
                    :, 128 + 32 * fb : 128 + 32 * (fb + 1), sh:X
                ]
        in_maps.append(
            {
                "ga": np.ascontiguousarray(gx[:, 0:128]),
                "gb": np.ascontiguousarray(gb23),
                "wt": wfull,
                "bias": b2,
            }
        )
    return in_maps


def run(x, weight, bias, idx, trace=False):
    idx = np.asarray(idx).astype(np.int64)
    assert idx.shape == (G_OUT, G_F) and idx.min() >= 0 and idx.max() < G_IN
    nc = _build_nc()
    in_maps = _prep_inputs(x, weight, bias, idx)
    res = run_bass_kernel_spmd(nc, in_maps, list(range(B)), trace=trace)
    out = np.stack([res.results[b]["out"] for b in range(B)]).astype(np.float32)
    return out, res


def kernel(x, weight, bias, idx):
    out, _ = run(x, weight, bias, idx, trace=False)
    return out


# revision 23
# speedup vs baseline: 1.1946x; 1.0038x over previous
"""Trainium2 Bass kernel for nn_GroupLocalSL2 — 23-stream variant.

out[b,o,i,xo,yo] = sum_{c,f,kh,kw} x[b,c,idx[i,f],xo+kh,yo+kw] * W[o,c,f,kh,kw] + bias[o]

Same skeleton as kernel.py (B=8 data-parallel, (c,f)-in-K, kw-pairs in M,
kh via PSUM accumulation), but the B-chunk (f=4..6, 96 rows) is repacked so
every B matmul streams a FULL 128-row contraction:

  The 15 B work-units (f in {4,5,6}) x (kh in 0..4), 32 rows each, pack into
  4 streams of 4 units instead of 5 streams of 3. Stream delta reads x rows
  r0+delta; a unit (f, kh) rides it iff its plane data is present shifted by
  (kh - delta) in {0, +1}. The +1-shifted copies are plain DMA loads of the
  same pre-gathered DRAM planes with a +1 source-row offset — no on-chip
  shuffles. Unit->stream map (fb = f-4):
     d0: (0,0)(1,0)(2,0)(0,1)   d1: (1,1)(2,1)(0,2)(1,2)
     d2: (2,2)(0,3)(1,3)(2,3)   d3: (0,4)(1,4)(2,4)(spare, zero weights)
  Phase-1 drops from 20 to 18 streams per row-chunk (25 -> 23 streams per
  output px; bf16 MAC floor is 21.875): ~45us less TensorE streaming.
  Cost: x-plane HBM traffic rises 2.8x (5.3MB/group), split across both
  HWDGE rings (sync: xa,xb0,xb1; scalar: xb2,xb3) so prefetch stays hidden.
"""

import os
import sys

import numpy as np
import ml_dtypes

for _p in ("/opt/trn_rl_repo", "/root/.axon_site/_ro/trn_rl_repo"):
    if os.path.isdir(_p) and _p not in sys.path:
        sys.path.append(_p)

import concourse.bass as bass
import concourse.mybir as mybir
import concourse.tile as tile
from concourse import bacc
from concourse.bass_utils import run_bass_kernel_spmd

BF16 = ml_dtypes.bfloat16

B, C, G_IN = 8, 32, 33
O, G_F, KH, KW = 64, 7, 5, 5
X, Y = 64, 64
G_OUT = 15
XO, YO = X - KH + 1, Y - KW + 1  # 60, 60
RCH = 8  # output rows per chunk (8*61 = 488 <= 512 psum bank)
N_WARM = 22  # dummy matmuls bridging program start to group-0 data-ready

# B-chunk unit map: BMAP[delta][slot] = (fb, kh); plane f=4+fb shifted by
# (kh - delta) rows lives at partitions slot*32:(slot+1)*32 of tile delta.
BMAP = {
    0: [(0, 0), (1, 0), (2, 0), (0, 1)],
    1: [(1, 1), (2, 1), (0, 2), (1, 2)],
    2: [(2, 2), (0, 3), (1, 3), (2, 3)],
    3: [(0, 4), (1, 4), (2, 4), None],
}


def _build_nc(n_groups=G_OUT):
    """Build the single-core Bass program (x pre-gathered host-side)."""
    nc = bacc.Bacc("TRN2", target_bir_lowering=False, debug=False)
    dt = mybir.dt
    ga_d = nc.dram_tensor("ga", [G_OUT, 128, X, Y + 1], dt.bfloat16, kind="ExternalInput")
    # the four B-tiles fully materialized host-side (shifted copies baked
    # in) so each is ONE contiguous DMA — HWDGE pays ~1.3us fixed per
    # transfer, so fewer/bigger transfers shorten the group-0 fill
    gb_d = nc.dram_tensor("gb", [G_OUT, 4, 128, X, Y + 1], dt.bfloat16, kind="ExternalInput")
    # packed weights: rows 0:5 = wa[kh], rows 5:10 = wb[kh] (kw4 odd half),
    # rows 10:14 = B-stream weights per delta (cols 0:256, kw-pair blocks)
    wt_d = nc.dram_tensor("wt", [128, 14, KW * O], dt.bfloat16, kind="ExternalInput")
    bias_d = nc.dram_tensor("bias", [O, 1], dt.float32, kind="ExternalInput")
    out_d = nc.dram_tensor("out", [O, G_OUT, XO, YO], dt.float32, kind="ExternalOutput")

    rchunks = [(r0, min(RCH, XO - r0)) for r0 in range(0, XO, RCH)]

    with tile.TileContext(nc) as tc:
        with (
            tc.tile_pool(name="wpool", bufs=1) as wpool,
            tc.tile_pool(name="warm", bufs=1) as warmpool,
            tc.tile_pool(name="xpool", bufs=2) as xpool,
            tc.tile_pool(name="tpool", bufs=3) as tpool,
            tc.tile_pool(name="opool", bufs=4) as opool,
            tc.tile_pool(name="psum", bufs=7, space="PSUM") as pp,
            tc.tile_pool(name="psumt", bufs=1, space="PSUM") as ppt,
        ):
            wmt = warmpool.tile([128, 256], dt.bfloat16, tag="warm")
            nc.gpsimd.memset(wmt[:, :], 0.0)

            # ONE whole-tile weight DMA: column-sliced weight transfers
            # degrade to 512B descriptors that clog the scalar ring for
            # ~10us (measured); the full tile is 8.9KB-contiguous per
            # partition. Tiny bias rides the sync chain at the very end.
            wt = wpool.tile([128, 14, KW * O], dt.bfloat16, tag="wt")
            bias_sb = wpool.tile([O, 1], dt.float32, tag="bias")
            nc.scalar.dma_start(wt[:, :, :], wt_d[:, :, :])

            for i in range(n_groups):
                xa = xpool.tile([128, X, Y + 1], dt.bfloat16, tag="xa")
                xbs = [
                    xpool.tile(
                        [128, X, Y + 1],
                        dt.bfloat16,
                        tag="xb%d" % d,
                        name="xb%d" % d,
                    )
                    for d in range(4)
                ]
                # one contiguous DMA per tile, split across both HWDGE rings;
                # group 0 in three row bands in consumption order so the
                # first row chunks' matmuls start ~15us earlier
                bands = ((0, 16), (16, 40), (40, X)) if i == 0 else ((0, X),)
                for lo, hi in bands:
                    nc.sync.dma_start(xa[:, lo:hi, :], ga_d[i, :, lo:hi, :])
                    nc.sync.dma_start(xbs[0][:, lo:hi, :], gb_d[i, 0, :, lo:hi, :])
                    nc.sync.dma_start(xbs[1][:, lo:hi, :], gb_d[i, 1, :, lo:hi, :])
                    nc.scalar.dma_start(xbs[2][:, lo:hi, :], gb_d[i, 2, :, lo:hi, :])
                    nc.scalar.dma_start(xbs[3][:, lo:hi, :], gb_d[i, 3, :, lo:hi, :])
                if i == 0:
                    nc.sync.dma_start(bias_sb[:, :], bias_d[:, :])
                if i == 0:
                    wps = ppt.tile([128, 4, 61], dt.float32, tag="pt")
                    for _ in range(N_WARM):
                        nc.tensor.matmul(
                            wps[:, :, :],
                            wmt[:, 0:128],
                            wmt[:, 0:244],
                            start=True,
                            stop=True,
                        )

                # Phase 1: per row chunk, 2 kw-pair groups x (5 A-streams +
                # 4 B-streams), all M=128 K=128, one psum bank per chunk.
                ptiles = []
                for r0, R in rchunks:
                    tail = R != RCH
                    p = (ppt if tail else pp).tile(
                        [128, R, 61], dt.float32, tag="pt" if tail else "p"
                    )
                    ptiles.append(p)
                    for grp in (0, 1):
                        c0 = 2 * grp
                        for kh in range(KH):
                            nc.tensor.matmul(
                                p[:, 0:R, :],
                                wt[0:128, kh, grp * 128 : grp * 128 + 128],
                                xa[0:128, r0 + kh : r0 + kh + R, c0 : c0 + 61],
                                start=(grp == 0 and kh == 0),
                                stop=False,
                            )
                        for dlt in range(4):
                            nc.tensor.matmul(
                                p[:, 0:R, :],
                                wt[0:128, 10 + dlt, grp * 128 : grp * 128 + 128],
                                xbs[dlt][0:128, r0 + dlt : r0 + dlt + R, c0 : c0 + 61],
                                start=False,
                                stop=False,
                            )
                for ri, ((r0, R), p) in enumerate(zip(rchunks, ptiles)):
                    # kw4 col-tiled pair (even: xa cols 4:65; odd: natural
                    # f4-6 at xb0[0:96], cols 3:64)
                    for kh in range(KH):
                        nc.tensor.matmul(
                            p[0:64, 0:R, :],
                            wt[0:128, kh, 256:320],
                            xa[0:128, r0 + kh : r0 + kh + R, 4:65],
                            start=False,
                            stop=False,
                        )
                        nc.tensor.matmul(
                            p[64:128, 0:R, :],
                            wt[0:96, KH + kh, 256:320],
                            xbs[0][0:96, r0 + kh : r0 + kh + R, 3:64],
                            start=False,
                            stop=(kh == KH - 1),
                        )

                    t = tpool.tile([O, RCH, 60], dt.float32, tag="t")
                    ot = opool.tile([O, RCH, 60], dt.float32, tag="out")
                    nc.scalar.add(t[:, 0:R, :], p[0:64, 0:R, 0:60], bias_sb[:, 0:1])
                    nc.vector.tensor_add(
                        ot[:, 0:R, :], t[:, 0:R, :], p[64:128, 0:R, 1:61]
                    )
                    if i == n_groups - 1 and ri == len(rchunks) - 1:
                        rh = max(R // 2, 1)
                        nc.sync.dma_start(
                            out_d[:, i, r0 : r0 + rh, :], ot[:, 0:rh, :]
                        )
                        nc.scalar.dma_start(
                            out_d[:, i, r0 + rh : r0 + R, :], ot[:, rh:R, :]
                        )
                    else:
                        nc.sync.dma_start(
                            out_d[:, i, r0 : r0 + R, :], ot[:, 0:R, :]
                        )
    nc.compile()
    return nc


def _prep_inputs(x, weight, bias, idx):
    """Host-side staging: bf16 cast, idx gather, packed lhsT weights."""
    x16 = np.asarray(x).astype(BF16)  # [B, C, G_IN, X, Y]
    x16 = np.pad(x16, ((0, 0), (0, 0), (0, 0), (0, 0), (0, 1)))
    w = np.asarray(weight).astype(np.float32)
    wx = w.transpose(2, 1, 3, 4, 0)  # [G_F, C, KH, KW, O]
    wa = wx[0:4].reshape(128, KH, KW * O)
    wb = wx[4:7].reshape(96, KH, KW * O)
    wfull = np.zeros((128, 14, KW * O), dtype=np.float32)
    wfull[:, 0:KH] = wa
    wfull[0:96, KH : 2 * KH] = wb
    for d, units in BMAP.items():
        for s, u in enumerate(units):
            if u is None:
                continue
            fb, kh = u
            wfull[32 * s : 32 * (s + 1), 10 + d, 0:256] = wx[
                4 + fb, :, kh, 0:4, :
            ].reshape(32, 256)
    wfull = np.ascontiguousarray(wfull).astype(BF16)
    b2 = np.ascontiguousarray(np.asarray(bias).astype(np.float32).reshape(O, 1))
    in_maps = []
    for b in range(B):
        gx = x16[b][:, idx]  # [C, G_OUT, G_F, X, Y+1]
        gx = gx.transpose(1, 2, 0, 3, 4).reshape(G_OUT, G_F * C, X, Y + 1)
        # materialize the four B-tiles (BMAP slots, shifts baked in)
        gb23 = np.zeros((G_OUT, 4, 128, X, Y + 1), dtype=BF16)
        for d, units in BMAP.items():
            for s, u in enumerate(units):
                if u is None:
                    continue
                fb, kh = u
                sh = kh - d
                gb23[:, d, 32 * s : 32 * (s + 1), 0 : X - sh] = gx[
                    :, 128 + 32 * fb : 128 + 32 * (fb + 1), sh:X
                ]
        in_maps.append(
            {
                "ga": np.ascontiguousarray(gx[:, 0:128]),
                "gb": np.ascontiguousarray(gb23),
                "wt": wfull,
                "bias": b2,
            }
        )
    return in_maps


def run(x, weight, bias, idx, trace=False):
    idx = np.asarray(idx).astype(np.int64)
    assert idx.shape == (G_OUT, G_F) and idx.min() >= 0 and idx.max() < G_IN
    nc = _build_nc()
    in_maps = _prep_inputs(x, weight, bias, idx)
    res = run_bass_kernel_spmd(nc, in_maps, list(range(B)), trace=trace)
    out = np.stack([res.results[b]["out"] for b in range(B)]).astype(np.float32)
    return out, res


def kernel(x, weight, bias, idx):
    out, _ = run(x, weight, bias, idx, trace=False)
    return out


# revision 28
# speedup vs baseline: 1.1966x; 1.0017x over previous
"""Trainium2 Bass kernel for nn_GroupLocalSL2 — 23-stream variant.

out[b,o,i,xo,yo] = sum_{c,f,kh,kw} x[b,c,idx[i,f],xo+kh,yo+kw] * W[o,c,f,kh,kw] + bias[o]

Same skeleton as kernel.py (B=8 data-parallel, (c,f)-in-K, kw-pairs in M,
kh via PSUM accumulation), but the B-chunk (f=4..6, 96 rows) is repacked so
every B matmul streams a FULL 128-row contraction:

  The 15 B work-units (f in {4,5,6}) x (kh in 0..4), 32 rows each, pack into
  4 streams of 4 units instead of 5 streams of 3. Stream delta reads x rows
  r0+delta; a unit (f, kh) rides it iff its plane data is present shifted by
  (kh - delta) in {0, +1}. The +1-shifted copies are plain DMA loads of the
  same pre-gathered DRAM planes with a +1 source-row offset — no on-chip
  shuffles. Unit->stream map (fb = f-4):
     d0: (0,0)(1,0)(2,0)(0,1)   d1: (1,1)(2,1)(0,2)(1,2)
     d2: (2,2)(0,3)(1,3)(2,3)   d3: (0,4)(1,4)(2,4)(spare, zero weights)
  Phase-1 drops from 20 to 18 streams per row-chunk (25 -> 23 streams per
  output px; bf16 MAC floor is 21.875): ~45us less TensorE streaming.
  Cost: x-plane HBM traffic rises 2.8x (5.3MB/group), split across both
  HWDGE rings (sync: xa,xb0,xb1; scalar: xb2,xb3) so prefetch stays hidden.
"""

import os
import sys

import numpy as np
import ml_dtypes

for _p in ("/opt/trn_rl_repo", "/root/.axon_site/_ro/trn_rl_repo"):
    if os.path.isdir(_p) and _p not in sys.path:
        sys.path.append(_p)

import concourse.bass as bass
import concourse.mybir as mybir
import concourse.tile as tile
from concourse import bacc
from concourse.bass_utils import run_bass_kernel_spmd

BF16 = ml_dtypes.bfloat16

B, C, G_IN = 8, 32, 33
O, G_F, KH, KW = 64, 7, 5, 5
X, Y = 64, 64
G_OUT = 15
XO, YO = X - KH + 1, Y - KW + 1  # 60, 60
RCH = 8  # output rows per chunk (8*61 = 488 <= 512 psum bank)
N_WARM = 22  # dummy matmuls bridging program start to group-0 data-ready

# B-chunk unit map: BMAP[delta][slot] = (fb, kh); plane f=4+fb shifted by
# (kh - delta) rows lives at partitions slot*32:(slot+1)*32 of tile delta.
BMAP = {
    0: [(0, 0), (1, 0), (2, 0), (0, 1)],
    1: [(1, 1), (2, 1), (0, 2), (1, 2)],
    2: [(2, 2), (0, 3), (1, 3), (2, 3)],
    3: [(0, 4), (1, 4), (2, 4), None],
}


def _build_nc(n_groups=G_OUT):
    """Build the single-core Bass program (x pre-gathered host-side)."""
    nc = bacc.Bacc("TRN2", target_bir_lowering=False, debug=False)
    dt = mybir.dt
    ga_d = nc.dram_tensor("ga", [G_OUT, 128, X, Y + 1], dt.bfloat16, kind="ExternalInput")
    # the four B-tiles fully materialized host-side (shifted copies baked
    # in) so each is ONE contiguous DMA — HWDGE pays ~1.3us fixed per
    # transfer, so fewer/bigger transfers shorten the group-0 fill
    gb_d = nc.dram_tensor("gb", [G_OUT, 4, 128, X, Y + 1], dt.bfloat16, kind="ExternalInput")
    # packed weights, kw-pair-group major so each group's block is DRAM-
    # contiguous and can load as one unfragmented DMA in consumption order:
    # rows 0:9 = grp0 (A kh0-4, B d0-3), rows 9:18 = grp1, rows 18:23 = kw4
    # (cols 0:64 even half, 64:128 odd half)
    wt_d = nc.dram_tensor("wt", [128, 23, 128], dt.bfloat16, kind="ExternalInput")
    bias_d = nc.dram_tensor("bias", [O, 1], dt.float32, kind="ExternalInput")
    out_d = nc.dram_tensor("out", [O, G_OUT, XO, YO], dt.float32, kind="ExternalOutput")

    rchunks = [(r0, min(RCH, XO - r0)) for r0 in range(0, XO, RCH)]

    with tile.TileContext(nc) as tc:
        with (
            tc.tile_pool(name="wpool", bufs=1) as wpool,
            tc.tile_pool(name="warm", bufs=1) as warmpool,
            tc.tile_pool(name="xpool", bufs=2) as xpool,
            tc.tile_pool(name="tpool", bufs=3) as tpool,
            tc.tile_pool(name="opool", bufs=4) as opool,
            tc.tile_pool(name="psum", bufs=7, space="PSUM") as pp,
            tc.tile_pool(name="psumt", bufs=1, space="PSUM") as ppt,
        ):
            wmt = warmpool.tile([128, 256], dt.bfloat16, tag="warm")
            nc.gpsimd.memset(wmt[:, :], 0.0)

            # three contiguous weight DMAs in consumption order (grp0 gates
            # matmul #1, grp1 gates #10, kw4 gates phase 2); each slice is
            # contiguous per partition, so no descriptor fragmentation.
            # Tiny bias rides the sync chain at the very end.
            wt = wpool.tile([128, 23, 128], dt.bfloat16, tag="wt")
            bias_sb = wpool.tile([O, 1], dt.float32, tag="bias")
            nc.scalar.dma_start(wt[:, 0:9, :], wt_d[:, 0:9, :])
            nc.scalar.dma_start(wt[:, 9:18, :], wt_d[:, 9:18, :])
            nc.scalar.dma_start(wt[:, 18:23, :], wt_d[:, 18:23, :])

            for i in range(n_groups):
                xa = xpool.tile([128, X, Y + 1], dt.bfloat16, tag="xa")
                xbs = [
                    xpool.tile(
                        [128, X, Y + 1],
                        dt.bfloat16,
                        tag="xb%d" % d,
                        name="xb%d" % d,
                    )
                    for d in range(4)
                ]
                # one contiguous DMA per tile, split across both HWDGE rings;
                # group 0 in three row bands in consumption order so the
                # first row chunks' matmuls start ~15us earlier
                bands = ((0, 16), (16, 40), (40, X)) if i == 0 else ((0, X),)
                for lo, hi in bands:
                    nc.sync.dma_start(xa[:, lo:hi, :], ga_d[i, :, lo:hi, :])
                    nc.sync.dma_start(xbs[0][:, lo:hi, :], gb_d[i, 0, :, lo:hi, :])
                    nc.sync.dma_start(xbs[1][:, lo:hi, :], gb_d[i, 1, :, lo:hi, :])
                    nc.scalar.dma_start(xbs[2][:, lo:hi, :], gb_d[i, 2, :, lo:hi, :])
                    nc.scalar.dma_start(xbs[3][:, lo:hi, :], gb_d[i, 3, :, lo:hi, :])
                if i == 0:
                    nc.sync.dma_start(bias_sb[:, :], bias_d[:, :])
                if i == 0:
                    wps = ppt.tile([128, 4, 61], dt.float32, tag="pt")
                    for _ in range(N_WARM):
                        nc.tensor.matmul(
                            wps[:, :, :],
                            wmt[:, 0:128],
                            wmt[:, 0:244],
                            start=True,
                            stop=True,
                        )

                # Phase 1: per row chunk, 2 kw-pair groups x (5 A-streams +
                # 4 B-streams), all M=128 K=128, one psum bank per chunk.
                ptiles = []
                for r0, R in rchunks:
                    tail = R != RCH
                    p = (ppt if tail else pp).tile(
                        [128, R, 61], dt.float32, tag="pt" if tail else "p"
                    )
                    ptiles.append(p)
                    # A-streams of BOTH kw-pair groups first (gated only on
                    # xa + the weight prefix), then B-streams delta-major —
                    # consumption order matches the head DMA arrival order
                    for grp in (0, 1):
                        for kh in range(KH):
                            nc.tensor.matmul(
                                p[:, 0:R, :],
                                wt[0:128, 9 * grp + kh, 0:128],
                                xa[0:128, r0 + kh : r0 + kh + R, 2 * grp : 2 * grp + 61],
                                start=(grp == 0 and kh == 0),
                                stop=False,
                            )
                    for dlt in range(4):
                        for grp in (0, 1):
                            nc.tensor.matmul(
                                p[:, 0:R, :],
                                wt[0:128, 9 * grp + 5 + dlt, 0:128],
                                xbs[dlt][0:128, r0 + dlt : r0 + dlt + R, 2 * grp : 2 * grp + 61],
                                start=False,
                                stop=False,
                            )
                for ri, ((r0, R), p) in enumerate(zip(rchunks, ptiles)):
                    # kw4 col-tiled pair (even: xa cols 4:65; odd: natural
                    # f4-6 at xb0[0:96], cols 3:64)
                    for kh in range(KH):
                        nc.tensor.matmul(
                            p[0:64, 0:R, :],
                            wt[0:128, 18 + kh, 0:64],
                            xa[0:128, r0 + kh : r0 + kh + R, 4:65],
                            start=False,
                            stop=False,
                        )
                        nc.tensor.matmul(
                            p[64:128, 0:R, :],
                            wt[0:96, 18 + kh, 64:128],
                            xbs[0][0:96, r0 + kh : r0 + kh + R, 3:64],
                            start=False,
                            stop=(kh == KH - 1),
                        )

                    t = tpool.tile([O, RCH, 60], dt.float32, tag="t")
                    ot = opool.tile([O, RCH, 60], dt.float32, tag="out")
                    nc.scalar.add(t[:, 0:R, :], p[0:64, 0:R, 0:60], bias_sb[:, 0:1])
                    nc.vector.tensor_add(
                        ot[:, 0:R, :], t[:, 0:R, :], p[64:128, 0:R, 1:61]
                    )
                    if i == n_groups - 1 and ri == len(rchunks) - 1:
                        rh = max(R // 2, 1)
                        nc.sync.dma_start(
                            out_d[:, i, r0 : r0 + rh, :], ot[:, 0:rh, :]
                        )
                        nc.scalar.dma_start(
                            out_d[:, i, r0 + rh : r0 + R, :], ot[:, rh:R, :]
                        )
                    else:
                        nc.sync.dma_start(
                            out_d[:, i, r0 : r0 + R, :], ot[:, 0:R, :]
                        )
    nc.compile()
    return nc


def _prep_inputs(x, weight, bias, idx):
    """Host-side staging: bf16 cast, idx gather, packed lhsT weights."""
    x16 = np.asarray(x).astype(BF16)  # [B, C, G_IN, X, Y]
    x16 = np.pad(x16, ((0, 0), (0, 0), (0, 0), (0, 0), (0, 1)))
    w = np.asarray(weight).astype(np.float32)
    wx = w.transpose(2, 1, 3, 4, 0)  # [G_F, C, KH, KW, O]
    wa = wx[0:4].reshape(128, KH, KW * O)
    wb = wx[4:7].reshape(96, KH, KW * O)
    # kw-pair-group-major packing: rows 0:9 grp0 (A kh0-4 + B d0-3),
    # rows 9:18 grp1, rows 18:23 kw4 (even | odd halves)
    wfull = np.zeros((128, 23, 128), dtype=np.float32)
    for grp in (0, 1):
        for kh in range(KH):
            wfull[:, 9 * grp + kh, :] = wa[:, kh, 128 * grp : 128 * (grp + 1)]
        for d, units in BMAP.items():
            for s, u in enumerate(units):
                if u is None:
                    continue
                fb, kh = u
                wfull[32 * s : 32 * (s + 1), 9 * grp + 5 + d, :] = wx[
                    4 + fb, :, kh, 2 * grp : 2 * grp + 2, :
                ].reshape(32, 128)
    for kh in range(KH):
        wfull[:, 18 + kh, 0:64] = wa[:, kh, 256:320]
        wfull[0:96, 18 + kh, 64:128] = wb[:, kh, 256:320]
    wfull = np.ascontiguousarray(wfull).astype(BF16)
    b2 = np.ascontiguousarray(np.asarray(bias).astype(np.float32).reshape(O, 1))
    in_maps = []
    for b in range(B):
        gx = x16[b][:, idx]  # [C, G_OUT, G_F, X, Y+1]
        gx = gx.transpose(1, 2, 0, 3, 4).reshape(G_OUT, G_F * C, X, Y + 1)
        # materialize the four B-tiles (BMAP slots, shifts baked in)
        gb23 = np.zeros((G_OUT, 4, 128, X, Y + 1), dtype=BF16)
        for d, units in BMAP.items():
            for s, u in enumerate(units):
                if u is None:
                    continue
                fb, kh = u
                sh = kh - d
                gb23[:, d, 32 * s : 32 * (s + 1), 0 : X - sh] = gx[
                    :, 128 + 32 * fb : 128 + 32 * (fb + 1), sh:X
                ]
        in_maps.append(
            {
                "ga": np.ascontiguousarray(gx[:, 0:128]),
                "gb": np.ascontiguousarray(gb23),
                "wt": wfull,
                "bias": b2,
            }
        )
    return in_maps


def run(x, weight, bias, idx, trace=False):
    idx = np.asarray(idx).astype(np.int64)
    assert idx.shape == (G_OUT, G_F) and idx.min() >= 0 and idx.max() < G_IN
    nc = _build_nc()
    in_maps = _prep_inputs(x, weight, bias, idx)
    res = run_bass_kernel_spmd(nc, in_maps, list(range(B)), trace=trace)
    out = np.stack([res.results[b]["out"] for b in range(B)]).astype(np.float32)
    return out, res


def kernel(x, weight, bias, idx):
    out, _ = run(x, weight, bias, idx, trace=False)
    return out
